# revision 1
# baseline (speedup 1.0000x reference)
"""Trainium2 Bass kernel for Gaussian KDE evaluation.

reference math:
    val[m] = (1/N) * sum_n exp(t1 - 0.5*d2(m,n)/bw^2)
    d2(m,n) = |e_m|^2 + |b_n|^2 - 2<e_m, b_n>
    t1 = -0.5*D*log(2*pi) - log_bw,  bw^2 = exp(2*log_bw)

Strategy (8 NeuronCores, x_eval row-sharded, x_base/log_bw replicated):
  Per core, one K=17 matmul per output tile produces |b|^2 - 2<e,b> in PSUM
  (stationary lhsT rows 0..15 = -2*eval^T, row 16 = ones; moving rhs rows
  0..15 = base^T, row 16 = |b|^2).  A single ScalarE ACTIVATE computes
  exp(scale*psum + bias) in place, with scale = -0.5/bw^2 and per-partition
  bias = t1 - ln(N) + scale*|e_m|^2, and its accum_out writes the row-sum.
  log_bw is broadcast on-device via a K=1 matmul; |b_n|^2 is moved from a
  per-partition column layout to a single-partition row via a DRAM bounce.
"""

import numpy as np

M, N, D = 8192, 16384, 16
NCORES = 8
MS = M // NCORES          # eval rows per core
RT = MS // 128            # row tiles per core (128 evals each)
CH = 1536                 # main column-chunk size (3 PSUM banks)
LOG_2PI = float(np.log(2.0 * np.pi))

_CACHE = {}


def _chunks():
    out = []
    c0 = 0
    while c0 < N:
        csz = min(CH, N - c0)
        out.append((c0, csz))
        c0 += csz
    return out


def _build_nc(reps=1, loop_iters=None, skip_act=False, skip_mm=False,
              skip_tp=False, max_chunks=None):
    from concourse import bacc, mybir, masks, tile

    f32 = mybir.dt.float32
    nc = bacc.Bacc("TRN2", target_bir_lowering=False, debug=False,
                   num_devices=NCORES)

    x_eval = nc.dram_tensor("x_eval", [MS, D], f32, kind="ExternalInput")
    x_base = nc.dram_tensor("x_base", [N, D], f32, kind="ExternalInput")
    log_bw = nc.dram_tensor("log_bw", [1, 1], f32, kind="ExternalInput")
    out = nc.dram_tensor("out", [128, RT], f32, kind="ExternalOutput")
    sqb_bounce = nc.dram_tensor("sqb_bounce", [1, N], f32)

    chunks = _chunks()
    NCH = len(chunks)
    NBT = N // 128            # number of 128-row base tiles
    Exp = mybir.ActivationFunctionType.Exp
    ADD = mybir.AluOpType.add
    MULT = mybir.AluOpType.mult
    X = mybir.AxisListType.X
    # constant part of the bias: t1 - ln(N) + log_bw-dependent part added
    # on-device; c0 covers everything except -log_bw and the |e|^2 term.
    c0 = -0.5 * D * LOG_2PI - float(np.log(N))

    with tile.TileContext(nc) as tc:
        with (
            tc.tile_pool(name="persist", bufs=1) as pp,
            tc.tile_pool(name="rhs", bufs=3) as rhsp,
            tc.tile_pool(name="mm", bufs=2, space="PSUM") as mmp,
            tc.tile_pool(name="tp", bufs=2, space="PSUM") as tpp,
        ):
          from contextlib import nullcontext
          for _rep in range(reps):
           with (tc.For_i(0, loop_iters, 1) if loop_iters else nullcontext()):
            identity = pp.tile([128, 128], f32)
            masks.make_identity(nc, identity[:])

            # ---- log_bw -> per-partition scale/bias columns -------------
            ones_row = pp.tile([1, 128], f32)
            nc.vector.memset(ones_row[:], 1.0)
            lb_sb = pp.tile([1, 1], f32)
            nc.sync.dma_start(out=lb_sb[:], in_=log_bw[:])
            ps_lb = tpp.tile([128, 512], f32, tag="tp")
            nc.tensor.matmul(ps_lb[:, 0:1], ones_row[:], lb_sb[:],
                             start=True, stop=True)
            # scale = -0.5 * exp(-2*log_bw)
            inv_bw2 = pp.tile([128, 1], f32)
            nc.scalar.activation(inv_bw2[:], ps_lb[:, 0:1], Exp, scale=-2.0)
            scale_col = pp.tile([128, 1], f32)
            nc.vector.tensor_scalar_mul(scale_col[:], inv_bw2[:], -0.5)
            # c_col = c0 - log_bw
            c_col = pp.tile([128, 1], f32)
            nc.vector.tensor_scalar(out=c_col[:], in0=ps_lb[:, 0:1],
                                    scalar1=-1.0, scalar2=c0,
                                    op0=MULT, op1=ADD)

            # ---- eval-side setup ----------------------------------------
            ev_nat = pp.tile([128, RT * D], f32)
            nc.sync.dma_start(
                out=ev_nat[:].rearrange("p (t d) -> p t d", d=D),
                in_=x_eval[:].rearrange("(p t) d -> p t d", p=128))
            ev_sq = pp.tile([128, RT * D], f32)
            nc.vector.tensor_mul(ev_sq[:], ev_nat[:], ev_nat[:])
            sq_e = pp.tile([128, RT], f32)
            nc.vector.tensor_reduce(
                out=sq_e[:], in_=ev_sq[:].rearrange("p (t d) -> p t d", d=D),
                axis=X, op=ADD)
            # bias_all[:, rt] = scale*|e|^2 + (c0 - log_bw)
            bias_all = pp.tile([128, RT], f32)
            nc.vector.tensor_scalar(out=bias_all[:], in0=sq_e[:],
                                    scalar1=scale_col[:, 0:1],
                                    scalar2=c_col[:, 0:1],
                                    op0=MULT, op1=ADD)

            # evT rows 0..15 = -2 * eval^T, row 16 = ones
            evT = pp.tile([17, MS], f32)
            nc.vector.memset(evT[:], 1.0)  # row 16 stays 1.0; rows 0..15 overwritten
            for rt in range(RT):
                ps_t = tpp.tile([16, 512], f32, tag="tp")
                nc.tensor.transpose(ps_t[:, 0:128],
                                    ev_nat[:, rt * D:(rt + 1) * D],
                                    identity[:])
                nc.vector.tensor_scalar_mul(
                    evT[0:16, rt * 128:(rt + 1) * 128], ps_t[:, 0:128], -2.0)

            # ---- base load + |b|^2 row (DRAM bounce) --------------------
            bs_nat = pp.tile([128, NBT * D], f32)
            nc.sync.dma_start(
                out=bs_nat[:].rearrange("p (t d) -> p t d", d=D),
                in_=x_base[:].rearrange("(p t) d -> p t d", p=128))
            bs_sq = pp.tile([128, NBT * D], f32)
            nc.vector.tensor_mul(bs_sq[:], bs_nat[:], bs_nat[:])
            sq_b = pp.tile([128, NBT], f32)
            nc.vector.tensor_reduce(
                out=sq_b[:], in_=bs_sq[:].rearrange("p (t d) -> p t d", d=D),
                axis=X, op=ADD)
            ps_sqb = tpp.tile([128, 512], f32, tag="tp")
            nc.tensor.transpose(ps_sqb[:, 0:128], sq_b[:], identity[:])
            sq_bT = pp.tile([128, 128], f32)
            nc.vector.tensor_copy(sq_bT[:], ps_sqb[:, 0:128])
            nc.sync.dma_start(
                out=sqb_bounce[:].rearrange("o (t p) -> (o t) p", p=128),
                in_=sq_bT[:])

            # ---- main loop ----------------------------------------------
            sums = pp.tile([128, RT * NCH], f32)
            if skip_act or (max_chunks is not None and max_chunks < NCH):
                nc.vector.memset(sums[:], 0.0)
            for ci, (cs, csz) in enumerate(chunks[:max_chunks]):
                rhs = rhsp.tile([17, CH], f32, tag="rhs")
                nt = csz // 128
                for g in range((nt + 3) // 4):
                    ps_t = tpp.tile([16, 512], f32, tag="tp")
                    for j in range(min(4, nt - 4 * g)):
                        t = cs // 128 + 4 * g + j
                        if not skip_tp:
                            nc.tensor.transpose(
                                ps_t[:, j * 128:(j + 1) * 128],
                                bs_nat[:, t * D:(t + 1) * D], identity[:])
                    w = min(512, (nt - 4 * g) * 128)
                    nc.vector.tensor_copy(
                        rhs[0:16, g * 512:g * 512 + w], ps_t[:, 0:w])
                nc.sync.dma_start(out=rhs[16:17, 0:csz],
                                  in_=sqb_bounce[0:1, cs:cs + csz])
                for rt in range(RT):
                    ps = mmp.tile([128, CH], f32, tag="mm")
                    if not skip_mm:
                        for j in range(csz // 512):
                            nc.tensor.matmul(
                                ps[:, j * 512:(j + 1) * 512],
                                evT[0:17, rt * 128:(rt + 1) * 128],
                                rhs[0:17, j * 512:(j + 1) * 512],
                                start=True, stop=True)
                    if not skip_act:
                        nc.scalar.activation(
                            ps[:, 0:csz], ps[:, 0:csz], Exp,
                            bias=bias_all[:, rt:rt + 1],
                            scale=scale_col[:, 0:1],
                            accum_out=sums[:, rt * NCH + ci:rt * NCH + ci + 1])

            # ---- finalize -----------------------------------------------
            val = pp.tile([128, RT], f32)
            for rt in range(RT):
                nc.vector.tensor_reduce(
                    out=val[:, rt:rt + 1],
                    in_=sums[:, rt * NCH:(rt + 1) * NCH], axis=X, op=ADD)
            nc.sync.dma_start(out=out[:], in_=val[:])

    nc.compile()
    return nc


def kernel(x_eval, x_base, log_bw):
    from concourse.bass_utils import run_bass_kernel_spmd

    if "nc" not in _CACHE:
        _CACHE["nc"] = _build_nc()
    nc = _CACHE["nc"]

    x_eval = np.ascontiguousarray(x_eval, dtype=np.float32)
    x_base = np.ascontiguousarray(x_base, dtype=np.float32)
    lb = np.asarray(log_bw, dtype=np.float32).reshape(1, 1)
    in_maps = [
        {
            "x_eval": x_eval[i * MS:(i + 1) * MS],
            "x_base": x_base,
            "log_bw": lb,
        }
        for i in range(NCORES)
    ]
    res = run_bass_kernel_spmd(nc, in_maps, list(range(NCORES)))
    # out[p, rt] holds eval point p*RT + rt of the shard -> row-major flatten
    shards = [r["out"].reshape(-1) for r in res.results]
    return np.concatenate(shards).astype(np.float32)



# revision 8
# speedup vs baseline: 2.5118x; 2.5118x over previous
"""Trainium2 Bass kernel for Gaussian KDE evaluation.

reference math:
    val[m] = (1/N) * sum_n exp(t1 - 0.5*d2(m,n)/bw^2)
    d2(m,n) = |e_m|^2 + |b_n|^2 - 2<e_m, b_n>
    t1 = -0.5*D*log(2*pi) - log_bw,  bw^2 = exp(2*log_bw)

Strategy (8 NeuronCores, x_eval row-sharded, x_base/log_bw replicated):
  The PE computes |b|^2 - 2<e,b> with bf16 operands split hi/lo for
  fp32-grade accuracy (compensated product: e.b ~ e_hi.b_hi + e_lo.b_hi
  + e_hi.b_lo, |b|^2 = sqb_hi + sqb_lo).  The terms are packed into one
  K=64 matmul per 128x512 output block:

    lhsT rows: [ hi(-2e^T) | lo(-2e^T) | hi(-2e^T) | 1 | 1 | 0-pad ]
    rhs  rows: [ hi(b^T)   | hi(b^T)   | lo(b^T)   | sqb_hi | sqb_lo | 0 ]

  Matmul cost is per *moving column*, so K=64 costs the same as K=17
  fp32 but streams at bf16 rate (1 PE cycle/col, 4x faster than fp32).

  The 64-row sections are built in natural [point, col] layout (free-dim
  writes have no partition-alignment constraint; compute-engine APs must
  start at partition multiples of 32), then one fp32 PE transpose flips
  TWO 64-col tile groups at once into PSUM, and a single DVE copy
  rounds them to bf16 in SBUF.  Per (row-tile, 2048-col chunk) a single
  ScalarE ACTIVATE computes exp(scale*psum + bias) over 4 PSUM banks
  (scale = -0.5/bw^2, bias = t1 - ln(N) + scale*|e|^2) and its
  accum_out emits the row-sum for free.  Two 4-bank PSUM tiles
  ping-pong between PE fill and ScalarE exp+accumulate.
"""

import numpy as np

M, N, D = 8192, 16384, 16
NCORES = 8
MS = M // NCORES          # eval rows per core
RT = MS // 128            # row tiles per core (128 evals each)
CH = 2048                 # base points per chunk (4 PSUM banks)
NCH = N // CH             # chunks per core
NBT = N // 128            # 128-row base tiles per core
SLABS = CH // 256         # PE transposes per chunk (2 tiles each)
LOG_2PI = float(np.log(2.0 * np.pi))

_CACHE = {}


def _build_nc():
    from concourse import bacc, mybir, masks, tile

    f32 = mybir.dt.float32
    bf16 = mybir.dt.bfloat16
    nc = bacc.Bacc("TRN2", target_bir_lowering=False, debug=False,
                   num_devices=NCORES)

    x_eval = nc.dram_tensor("x_eval", [MS, D], f32, kind="ExternalInput")
    x_base = nc.dram_tensor("x_base", [N, D], f32, kind="ExternalInput")
    log_bw = nc.dram_tensor("log_bw", [1, 1], f32, kind="ExternalInput")
    out = nc.dram_tensor("out", [128, RT], f32, kind="ExternalOutput")

    Exp = mybir.ActivationFunctionType.Exp
    ADD = mybir.AluOpType.add
    SUB = mybir.AluOpType.subtract
    MULT = mybir.AluOpType.mult
    X = mybir.AxisListType.X
    # constant bias: t1 - ln(N); -log_bw and scale*|e|^2 added on-device
    c0 = -0.5 * D * LOG_2PI - float(np.log(N))

    with tile.TileContext(nc) as tc:
        with (
            tc.tile_pool(name="persist", bufs=1) as pp,
            tc.tile_pool(name="rhs", bufs=2) as rhsp,
            tc.tile_pool(name="mm", bufs=2, space="PSUM") as mmp,
        ):
            identity = pp.tile([128, 128], f32)
            masks.make_identity(nc, identity[:])

            setup_ps = mmp.tile([128, CH], f32, tag="mm")

            # ---- log_bw -> per-partition scale/bias columns -------------
            ones_row = pp.tile([1, 128], f32)
            nc.vector.memset(ones_row[:], 1.0)
            lb_sb = pp.tile([1, 1], f32)
            nc.sync.dma_start(out=lb_sb[:], in_=log_bw[:])
            nc.tensor.matmul(setup_ps[:, 1536:1537], ones_row[:], lb_sb[:],
                             start=True, stop=True)
            # scale = -0.5 * exp(-2*log_bw)
            inv_bw2 = pp.tile([128, 1], f32)
            nc.scalar.activation(inv_bw2[:], setup_ps[:, 1536:1537], Exp,
                                 scale=-2.0)
            scale_col = pp.tile([128, 1], f32)
            nc.vector.tensor_scalar_mul(scale_col[:], inv_bw2[:], -0.5)
            # c_col = c0 - log_bw
            c_col = pp.tile([128, 1], f32)
            nc.vector.tensor_scalar(out=c_col[:], in0=setup_ps[:, 1536:1537],
                                    scalar1=-1.0, scalar2=c0,
                                    op0=MULT, op1=ADD)

            # ---- eval-side setup ----------------------------------------
            ev_nat = pp.tile([128, RT * D], f32)
            nc.sync.dma_start(
                out=ev_nat[:].rearrange("p (t d) -> p t d", d=D),
                in_=x_eval[:].rearrange("(p t) d -> p t d", p=128))
            ev_sq = pp.tile([128, RT * D], f32)
            nc.vector.tensor_mul(ev_sq[:], ev_nat[:], ev_nat[:])
            sq_e = pp.tile([128, RT], f32)
            nc.vector.tensor_reduce(
                out=sq_e[:], in_=ev_sq[:].rearrange("p (t d) -> p t d", d=D),
                axis=X, op=ADD)
            # bias_all[:, rt] = scale*|e|^2 + (c0 - log_bw)
            bias_all = pp.tile([128, RT], f32)
            nc.vector.tensor_scalar(out=bias_all[:], in0=sq_e[:],
                                    scalar1=scale_col[:, 0:1],
                                    scalar2=c_col[:, 0:1],
                                    op0=MULT, op1=ADD)

            # ev_ext64[:, rt, :]: cols 0-15 hi(-2e), 16-31 lo(-2e),
            # 32-47 hi(-2e), 48 = 49 = 1.0, 50-63 zero pad
            m2e = pp.tile([128, RT * D], f32)
            nc.vector.tensor_scalar_mul(m2e[:], ev_nat[:], -2.0)
            ev_hi = pp.tile([128, RT * D], bf16)
            nc.vector.tensor_copy(ev_hi[:], m2e[:])
            ev_ext = pp.tile([128, RT * 64], f32)
            nc.vector.memset(ev_ext[:], 0.0)
            evv = ev_ext[:].rearrange("p (t s) -> p t s", s=64)
            ev_hi3 = ev_hi[:].rearrange("p (t d) -> p t d", d=D)
            m2e3 = m2e[:].rearrange("p (t d) -> p t d", d=D)
            nc.vector.tensor_copy(evv[:, :, 0:16], ev_hi3)
            nc.vector.tensor_tensor(out=evv[:, :, 16:32], in0=m2e3,
                                    in1=ev_hi3, op=SUB)
            nc.vector.tensor_copy(evv[:, :, 32:48], ev_hi3)
            nc.vector.memset(evv[:, :, 48:50], 1.0)
            for rt in range(RT):
                nc.tensor.transpose(setup_ps[0:64, rt * 128:(rt + 1) * 128],
                                    ev_ext[:, rt * 64:(rt + 1) * 64],
                                    identity[:])
            # rows 64-127 duplicate rows 0-63: matmul operands must share a
            # base partition, so the B-half matmuls read evT[64:128]
            evT = pp.tile([128, MS], bf16)
            nc.vector.tensor_copy(evT[0:64, :], setup_ps[0:64, 0:MS])
            nc.vector.tensor_copy(evT[64:128, :], evT[0:64, :])

            # ---- base-side setup ----------------------------------------
            # bs_ext64[:, t, :]: cols 0-15 hi(b), 16-31 hi(b), 32-47 lo(b),
            # 48 sqb_hi, 49 sqb_lo, 50-63 zero pad   (f32 values; the
            # hi parts are bf16-representable, rounded on the PSUM->SBUF
            # bf16 copy)
            bs_nat = pp.tile([128, NBT * D], f32)
            nc.sync.dma_start(
                out=bs_nat[:].rearrange("p (t d) -> p t d", d=D),
                in_=x_base[:].rearrange("(p t) d -> p t d", p=128))
            bs_hi = pp.tile([128, NBT * D], bf16)
            nc.vector.tensor_copy(bs_hi[:], bs_nat[:])
            bs_sq = pp.tile([128, NBT * D], f32)
            nc.vector.tensor_mul(bs_sq[:], bs_nat[:], bs_nat[:])
            sq_b = pp.tile([128, NBT], f32)
            nc.vector.tensor_reduce(
                out=sq_b[:], in_=bs_sq[:].rearrange("p (t d) -> p t d", d=D),
                axis=X, op=ADD)
            sqb_hi = pp.tile([128, NBT], bf16)
            nc.vector.tensor_copy(sqb_hi[:], sq_b[:])
            bs_ext = pp.tile([128, NBT * 64], f32)
            nc.vector.memset(bs_ext[:], 0.0)
            bsv = bs_ext[:].rearrange("p (t s) -> p t s", s=64)
            bs_hi3 = bs_hi[:].rearrange("p (t d) -> p t d", d=D)
            bs_nat3 = bs_nat[:].rearrange("p (t d) -> p t d", d=D)
            nc.vector.tensor_copy(bsv[:, :, 0:16], bs_hi3)
            nc.vector.tensor_copy(bsv[:, :, 16:32], bs_hi3)
            nc.vector.tensor_tensor(out=bsv[:, :, 32:48], in0=bs_nat3,
                                    in1=bs_hi3, op=SUB)
            sqb_hi3 = sqb_hi[:].rearrange("p (t o) -> p t o", o=1)
            sq_b3 = sq_b[:].rearrange("p (t o) -> p t o", o=1)
            nc.vector.tensor_copy(bsv[:, :, 48:49], sqb_hi3)
            nc.vector.tensor_tensor(out=bsv[:, :, 49:50], in0=sq_b3,
                                    in1=sqb_hi3, op=SUB)

            # ---- main loop ----------------------------------------------
            sums = pp.tile([128, RT * NCH], f32)
            for ci in range(NCH):
                tA = mmp.tile([128, CH], f32, tag="mm")
                tB = mmp.tile([128, CH], f32, tag="mm")
                rhs = rhsp.tile([128, CH // 2], bf16, tag="rhs")
                # stage two 64-section tile groups per fp32 transpose
                for s in range(SLABS):
                    nc.tensor.transpose(
                        tA[:, s * 128:(s + 1) * 128],
                        bs_ext[:, (ci * SLABS + s) * 128:
                               (ci * SLABS + s + 1) * 128],
                        identity[:])
                nc.vector.tensor_copy(rhs[:], tA[:, 0:CH // 2])
                for rt in range(RT):
                    ps = tA if rt % 2 == 0 else tB
                    lhsA = evT[0:64, rt * 128:(rt + 1) * 128]
                    lhsB = evT[64:128, rt * 128:(rt + 1) * 128]
                    half = CH // 4      # 512
                    nc.tensor.matmul(ps[:, 0:half], lhsA,
                                     rhs[0:64, 0:half],
                                     start=True, stop=True)
                    nc.tensor.matmul(ps[:, half:2 * half], lhsA,
                                     rhs[0:64, half:2 * half],
                                     start=True, stop=True)
                    nc.tensor.matmul(ps[:, 2 * half:3 * half], lhsB,
                                     rhs[64:128, 0:half],
                                     start=True, stop=True)
                    nc.tensor.matmul(ps[:, 3 * half:4 * half], lhsB,
                                     rhs[64:128, half:2 * half],
                                     start=True, stop=True)
                    nc.scalar.activation(
                        ps[:, :], ps[:, :], Exp,
                        bias=bias_all[:, rt:rt + 1],
                        scale=scale_col[:, 0:1],
                        accum_out=sums[:, rt * NCH + ci:rt * NCH + ci + 1])

            # ---- finalize -----------------------------------------------
            val = pp.tile([128, RT], f32)
            nc.vector.tensor_reduce(
                out=val[:],
                in_=sums[:].rearrange("p (r c) -> p r c", c=NCH),
                axis=X, op=ADD)
            nc.sync.dma_start(out=out[:], in_=val[:])

    nc.compile()
    return nc


def kernel(x_eval, x_base, log_bw):
    from concourse.bass_utils import run_bass_kernel_spmd

    if "nc" not in _CACHE:
        _CACHE["nc"] = _build_nc()
    nc = _CACHE["nc"]

    x_eval = np.ascontiguousarray(x_eval, dtype=np.float32)
    x_base = np.ascontiguousarray(x_base, dtype=np.float32)
    lb = np.asarray(log_bw, dtype=np.float32).reshape(1, 1)
    in_maps = [
        {
            "x_eval": x_eval[i * MS:(i + 1) * MS],
            "x_base": x_base,
            "log_bw": lb,
        }
        for i in range(NCORES)
    ]
    res = run_bass_kernel_spmd(nc, in_maps, list(range(NCORES)))
    # out[p, rt] holds eval point p*RT + rt of the shard -> row-major flatten
    shards = [r["out"].reshape(-1) for r in res.results]
    return np.concatenate(shards).astype(np.float32)
